# revision 20
# baseline (speedup 1.0000x reference)
"""Trainium2 Bass kernel for BatchAll triplet loss.

Reference computation (B=512, D=1024):
    pw = img @ sent.T                                  [B, B]
    t[a,p,n] = pw[a,p] - pw[a,n] + margin
    valid[a,p,n] = (lab[a]==lab[p]) & (lab[a]!=lab[n])
    loss = sum(relu(valid*t)) / (count(valid*t > EPS) + EPS)

Strategy: the batch is class-sorted on the host (a pure permutation of the
(image, sentence, label) triples; the loss is permutation invariant), then
anchors are sharded across 8 cores (64 each, C = core*64). After sorting,
all positives of anchor a live in a contiguous class run inside the core's
128-wide sentence window [C-32, C+96) (holds when max class size <= 33;
dense fallback otherwise). Each core enumerates its actual valid (a,p)
pairs (sum of class sizes over its anchors, ~320 for uniform labels) and
packs them onto partitions: tiles of 128 pairs, free axis = all 512 n.

Per core, with the sentence axis pre-rotated so the window is cols [0,128):
    pw[a,n]   (PE, fp8e5m2 DoubleRow, 4 matmuls)         [64, 512]
    zext[a,n] = pw[a,n] - margin  (UNMASKED, fp16; row 64 = +30000 pad)
    per pair-tile t:
        Z[k,n]  = zext[a_k, n]        (PE one-hot matmul)    [128, 512]
        w[k]    = (Z[k,0:W]+margin) . oneJ  (DVE rowsum)  = pw[a_k,p_k]
        relu(-Z + w) accum -> Sacc    (ACT; includes same-label n)
        count Z < w accum  -> Cacc    (Pool; includes same-label n)
        corrections over the 128-wide window only (same-label n live
        there): relu(w-Z)*eq accum -> corrS, (Z<w)*eq accum -> corrC
Host reduces: loss = (S - corrS) / (C - corrC + EPS).

All DMA rides the single sync HWDGE queue (one queue-drain at exit);
packT is pre-packed partition-major so each partition is one contiguous
descriptor. Raw [128, T]-ish accumulators are DMA'd out; host reduces.
"""

import numpy as np
from contextlib import ExitStack

B = 512
D = 1024
NCORES = 8
A = B // NCORES   # 64 anchors per core
KT = D // 128     # 8 contraction tiles
NT = B // 128     # 4 n-tiles per anchor (dense variant)
W = 128           # per-core sentence window width
MARGIN = 0.2
EPS = 1e-16
BIG = 1e30
BIGW = 30000.0
MAXC_WIN = 33     # pair variant valid iff max class size <= this
FP8 = True        # embeddings in fp8e5m2: halves the packT DMA

_CACHE = {}


def _build_pairs(T):
    """Pair-packed kernel, raw bass (no TileContext): explicit semaphores,
    no framework drain/teardown. T tiles of 128 (anchor, positive) pairs."""
    import concourse.mybir as mybir
    from concourse import bacc

    f32 = mybir.dt.float32
    f16 = mybir.dt.float16
    Alu = mybir.AluOpType
    Act = mybir.ActivationFunctionType

    nc = bacc.Bacc("TRN2", target_bir_lowering=False, debug=False,
                   num_devices=NCORES)

    f8 = mybir.dt.float8e5
    packT_d = nc.dram_tensor("packT", [128, KT, A + B], f8 if FP8 else f16,
                             kind="ExternalInput")
    # aux regions along dim1: [0,T) oneJ | [T,2T) eqwin | [2T,3T) selA
    aux_d = nc.dram_tensor("aux", [128, 3 * T, W], f16, kind="ExternalInput")
    # out cols: [0,T) Sacc | [T,2T) count | [2T,3T) corrS | [3T,4T) corrC
    out_d = nc.dram_tensor("out", [128, 4 * T], f32, kind="ExternalOutput")

    from contextlib import ExitStack
    with ExitStack() as ctx:
        def sb(name, shape, dt):
            return ctx.enter_context(nc.sbuf_tensor(name, shape, dt))

        def ps(name, shape, dt):
            return ctx.enter_context(nc.psum_tensor(name, shape, dt))

        def sem(name):
            return ctx.enter_context(nc.semaphore(name))

        packT = sb("packT_s", [128, KT, A + B], f8 if FP8 else f16)
        aux = sb("aux_s", [128, 3 * T, W], f16)
        zext = sb("zext", [A + 1, B], f16)
        wcol = sb("wcol", [128, T], f32)
        SC = sb("SC", [128, 4 * T], f32)
        r_all = sb("r_all", [128, T, B], f16)
        junk = [sb(f"junk{t}", [128, W], f16) for t in range(T)]
        mj = [sb(f"mj{t}", [128, B], f16) for t in range(T)]
        csj = [sb(f"csj{t}", [128, W], f16) for t in range(T)]
        ccj = [sb(f"ccj{t}", [128, W], f16) for t in range(T)]
        dum = sb("dum", [1, 1], f16)

        pw_ps = ps("pw_ps", [A, B], f32)
        z_ps = [ps(f"z_ps{t}", [128, B], f32) for t in range(T)]

        sq1 = sem("sq1")    # sync HWDGE: packT k01
        sq2 = sem("sq2")    # sync HWDGE: packT k45
        sa1 = sem("sa1")    # scalar HWDGE: packT k23
        sa2 = sem("sa2")    # scalar HWDGE: packT k67
        so = sem("so")      # sync HWDGE: out
        sw = sem("sw")      # gpsimd SWDGE: aux
        spad = sem("spad")  # zext pad row
        spw = sem("spw")    # pw matmuls done
        szx = sem("szx")    # zext rows written
        sz = sem("sz")      # z matmuls (+1 each)
        swc = sem("swc")    # wcol cols (+1 each)
        sr = sem("sr")      # relus done (+1 each)
        sv = sem("sv")      # vector post-ops done

        with nc.Block(no_gpsimd_drain=True) as block:

            @block.sync
            def _(sync):
                sync.dma_start(out=packT[:, 0:2, :],
                               in_=packT_d.ap()[:, 0:2, :]).then_inc(sq1, 16)
                sync.dma_start(out=packT[:, 4:6, :],
                               in_=packT_d.ap()[:, 4:6, :]).then_inc(sq2, 16)
                sync.wait_ge(sr, 3)
                sync.wait_ge(sv, 1)
                sync.dma_start(out=out_d.ap(), in_=SC[:, :],
                               single_packet=True).then_inc(so, 16)
                sync.wait_ge(so, 16)

            @block.gpsimd
            def _(gpsimd):
                gpsimd.dma_start(out=aux[:, :, :], in_=aux_d.ap()).then_inc(sw, 16)
                gpsimd.memset(zext[A:A + 1, :], BIGW).then_inc(spad, 1)

            @block.tensor
            def _(tensor):
                # chunk order k01(sync) k23(scalar) k45(sync) k67(scalar):
                # each chunk's completion crawl overlaps the previous matmul
                waits = [(sq1, 16), (sa1, 16), (sq2, 16), (sa2, 16)]
                chunks = [0, 1, 2, 3]
                for i, u in enumerate(chunks):
                    tensor.wait_ge(*waits[i])
                    mm = tensor.matmul(
                        pw_ps[:, :], lhsT=packT[:, 2 * u:2 * u + 2, 0:A],
                        rhs=packT[:, 2 * u:2 * u + 2, A:A + B],
                        start=(i == 0), stop=(i == 3),
                        perf_mode=mybir.MatmulPerfMode.DoubleRow)
                    if i == 3:
                        mm.then_inc(spw, 1)
                tensor.wait_ge(szx, 2)
                tensor.wait_ge(spad, 1)
                tensor.wait_ge(sw, 16)
                for t in range(T):
                    tensor.matmul(z_ps[t][:, :], lhsT=aux[0:A + 1, 2 * T + t, :],
                                  rhs=zext[:, :]).then_inc(sz, 1)

            @block.scalar
            def _(scalar):
                scalar.dma_start(out=packT[:, 2:4, :],
                                 in_=packT_d.ap()[:, 2:4, :]).then_inc(sa1, 16)
                scalar.dma_start(out=packT[:, 6:8, :],
                                 in_=packT_d.ap()[:, 6:8, :]).then_inc(sa2, 16)
                # dummy relu pulls the ACT table load off the critical path
                scalar.wait_ge(spad, 1)
                scalar.activation(out=dum[:, :], in_=zext[A:A + 1, 0:1],
                                  func=Act.Relu, bias=0.0, scale=1.0)
                scalar.wait_ge(spw, 1)
                scalar.activation(out=zext[0:A // 2, :],
                                  in_=pw_ps[:, :][0:A // 2, :], func=Act.Copy,
                                  bias=-MARGIN, scale=1.0).then_inc(szx, 1)
                for t in range(T):
                    scalar.wait_ge(sz, t + 1)
                    scalar.wait_ge(swc, t + 1)
                    scalar.activation(
                        out=r_all[:, t, :], in_=z_ps[t][:, :], func=Act.Relu,
                        bias=wcol[:, t:t + 1], scale=-1.0,
                        accum_out=SC[:, t:t + 1]).then_inc(sr, 1)

            @block.vector
            def _(vector):
                vector.wait_ge(spw, 1)
                vector.tensor_scalar(
                    zext[A // 2:A, :], pw_ps[:, :][A // 2:A, :], -MARGIN,
                    None, Alu.add).then_inc(szx, 1)
                vector.wait_ge(sw, 16)
                for t in range(T):
                    vector.wait_ge(sz, t + 1)
                    vector.scalar_tensor_tensor(
                        junk[t][:, :], z_ps[t][:, 0:W], MARGIN, aux[:, t, :],
                        Alu.add, Alu.mult,
                        accum_out=wcol[:, t:t + 1]).then_inc(swc, 1)
                for t in range(T):
                    vector.wait_ge(sr, t + 1)
                    vector.tensor_scalar(
                        mj[t][:, :], r_all[:, t, :], 0.0, None, Alu.is_gt,
                        Alu.add, accum_out=SC[:, T + t:T + t + 1])
                    vector.scalar_tensor_tensor(
                        csj[t][:, :], r_all[:, t, 0:W], 1.0,
                        aux[:, T + t, :], Alu.mult, Alu.mult,
                        accum_out=SC[:, 2 * T + t:2 * T + t + 1])
                    cc = vector.scalar_tensor_tensor(
                        ccj[t][:, :], r_all[:, t, 0:W], 0.0,
                        aux[:, T + t, :], Alu.is_gt, Alu.mult,
                        accum_out=SC[:, 3 * T + t:3 * T + t + 1])
                    if t == T - 1:
                        cc.then_inc(sv, 1)

        nc.compile()
    return nc


def _build_dense():
    """Dense fallback (no class-size assumption)."""
    import concourse.mybir as mybir
    import concourse.tile as tile
    from concourse import bacc
    from concourse.masks import make_identity

    f32 = mybir.dt.float32
    bf16 = mybir.dt.bfloat16
    Alu = mybir.AluOpType
    Act = mybir.ActivationFunctionType
    Ax = mybir.AxisListType

    nc = bacc.Bacc("TRN2", target_bir_lowering=False, debug=False,
                   num_devices=NCORES)

    imgT_d = nc.dram_tensor("imgT", [D, A], f32, kind="ExternalInput")
    sentT_d = nc.dram_tensor("sentT", [D, B], f32, kind="ExternalInput")
    labf_d = nc.dram_tensor("labf", [B], bf16, kind="ExternalInput")
    labc_d = nc.dram_tensor("labc", [A], f32, kind="ExternalInput")
    out_d = nc.dram_tensor("out", [2], f32, kind="ExternalOutput")

    with tile.TileContext(nc) as tc:
        with ExitStack() as ctx:
            singles = ctx.enter_context(tc.tile_pool(name="singles", bufs=1))
            rpool = ctx.enter_context(tc.tile_pool(name="rpool", bufs=6))
            mpool = ctx.enter_context(tc.tile_pool(name="mpool", bufs=6))
            spsum = ctx.enter_context(
                tc.tile_pool(name="spsum", bufs=1, space="PSUM"))
            wpsum = ctx.enter_context(
                tc.tile_pool(name="wpsum", bufs=3, space="PSUM"))
            gpsum = ctx.enter_context(
                tc.tile_pool(name="gpsum", bufs=2, space="PSUM"))

            ones_r = singles.tile([1, 128], f32)
            nc.vector.memset(ones_r, 1.0)
            ones_c = singles.tile([128, 1], f32)
            nc.vector.memset(ones_c, 1.0)
            ident = singles.tile([64, 64], f32)
            make_identity(nc, ident)

            imgT = singles.tile([128, KT, A], f32)
            nc.sync.dma_start(
                out=imgT, in_=imgT_d.ap().rearrange("(t p) m -> p t m", p=128))
            sentT = singles.tile([128, KT, B], f32)
            nc.sync.dma_start(
                out=sentT, in_=sentT_d.ap().rearrange("(t p) m -> p t m", p=128))
            lab_row = singles.tile([1, B], f32)
            nc.sync.dma_start(
                out=lab_row, in_=labf_d.ap().rearrange("(o b) -> o b", o=1))
            labc_col = singles.tile([A, 1], f32)
            nc.sync.dma_start(
                out=labc_col, in_=labc_d.ap().rearrange("(a o) -> a o", o=1))

            pw_ps = spsum.tile([A, B], f32)
            for kt in range(KT):
                nc.tensor.matmul(pw_ps, lhsT=imgT[:, kt, :], rhs=sentT[:, kt, :],
                                 start=(kt == 0), stop=(kt == KT - 1))

            labB_ps = spsum.tile([A, B], f32)
            nc.tensor.matmul(labB_ps, lhsT=ones_r[:, :A], rhs=lab_row)
            eqP = singles.tile([A, B], f32)
            nc.vector.tensor_scalar(eqP, labB_ps, labc_col, None, Alu.is_equal)
            penP = singles.tile([A, B], f32)
            nc.vector.tensor_scalar(penP, eqP, 1.0, BIG, Alu.subtract, Alu.mult)
            penN = singles.tile([A, B], f32)
            nc.vector.tensor_scalar(penN, eqP, -BIG, None, Alu.mult)

            w = singles.tile([A, B], f32)
            nc.vector.tensor_scalar(w, pw_ps, MARGIN, None, Alu.add)
            nc.vector.tensor_mul(w, w, eqP)
            nc.vector.tensor_add(w, w, penP)
            negneq = singles.tile([A, B], f32)
            nc.vector.tensor_scalar(negneq, eqP, 1.0, -1.0, Alu.subtract,
                                    Alu.mult)
            z = singles.tile([A, B], f32)
            nc.vector.tensor_scalar(z, pw_ps, -1.0, None, Alu.mult)
            nc.vector.tensor_mul(z, z, negneq)
            nc.vector.tensor_add(z, z, penN)

            zTs = singles.tile([128, NT, A], f32)
            for j in range(NT):
                zt_ps = spsum.tile([128, A], f32)
                nc.tensor.transpose(zt_ps, z[:, j * 128:(j + 1) * 128], ident)
                nc.scalar.copy(zTs[:, j, :], zt_ps)

            Sacc = singles.tile([128, A * NT], f32)
            Cacc = singles.tile([128, A * NT], f32)

            for a in range(A):
                wb_ps = wpsum.tile([128, B], f32)
                nc.tensor.matmul(
                    wb_ps, lhsT=ident[:, a:a + 1].broadcast_to([A, 128]), rhs=w)
                for j in range(NT):
                    col = a * NT + j
                    r = rpool.tile([128, B], bf16)
                    nc.scalar.activation(
                        out=r, in_=wb_ps, func=Act.Relu,
                        bias=zTs[:, j, a:a + 1], scale=1.0,
                        accum_out=Sacc[:, col:col + 1])
                    m = mpool.tile([128, B], bf16)
                    nc.vector.tensor_scalar(
                        m, r, EPS, None, Alu.is_gt, Alu.add,
                        accum_out=Cacc[:, col:col + 1])

            SC = singles.tile([128, 2], f32)
            nc.vector.tensor_reduce(SC[:, 0:1], Sacc, Ax.X, Alu.add)
            nc.vector.tensor_reduce(SC[:, 1:2], Cacc, Ax.X, Alu.add)
            fin_ps = spsum.tile([2, 1], f32)
            nc.tensor.matmul(fin_ps, lhsT=SC, rhs=ones_c)
            fin_sb = singles.tile([2, 1], f32)
            nc.scalar.copy(fin_sb, fin_ps)
            nc.sync.dma_start(
                out=out_d.ap().rearrange("(p o) -> p o", o=1), in_=fin_sb)

    nc.compile()
    return nc


def _get_nc(variant, T=0):
    key = f"nc_{variant}_{T}"
    if key not in _CACHE:
        _CACHE[key] = (_build_pairs(T) if variant == "pairs"
                       else _build_dense())
    return _CACHE[key]


def _prep(labels, image_embeddings, sentence_embeddings):
    """Class-sort the batch; build per-core input maps."""
    labels = np.ascontiguousarray(labels).astype(np.int64)
    img = np.ascontiguousarray(image_embeddings, dtype=np.float32)
    sent = np.ascontiguousarray(sentence_embeddings, dtype=np.float32)
    counts = np.bincount(labels, minlength=1)
    maxc = counts.max()

    perm = np.argsort(labels, kind="stable")
    labs = labels[perm]

    if maxc > MAXC_WIN:
        imgT = np.ascontiguousarray(img[perm].T)    # [D, B]
        sentT = np.ascontiguousarray(sent[perm].T)  # [D, B]
        labsf = labs.astype(np.float32)
        maps = []
        for i in range(NCORES):
            c0 = i * A
            maps.append({
                "imgT": np.ascontiguousarray(imgT[:, c0:c0 + A]),
                "sentT": sentT,
                "labf": labsf,
                "labc": np.ascontiguousarray(labsf[c0:c0 + A]),
            })
        return "dense", 0, maps

    if FP8:
        import ml_dtypes
        edt = ml_dtypes.float8_e5m2
    else:
        edt = np.float16
    imgT = np.ascontiguousarray(img[perm].T).astype(edt)
    sentT = np.ascontiguousarray(sent[perm].T).astype(edt)

    # class run start/size per sorted position
    starts = np.concatenate([[0], np.cumsum(counts)])
    s_a = starts[labs]            # run start of each anchor
    n_a = counts[labs]            # run length of each anchor
    maxK = max(int(n_a[c0:c0 + A].sum()) for c0 in range(0, B, A))
    T = (maxK + 127) // 128
    K = T * 128

    maps = []
    for i in range(NCORES):
        c0 = i * A
        rot = (np.arange(B) + c0 - 32) % B
        # partition-major packT: [p, t, m] = flat[(t*128+p), m]
        flat = np.concatenate([imgT[:, c0:c0 + A], sentT[:, rot]], axis=1)
        packT = np.ascontiguousarray(
            flat.reshape(KT, 128, A + B).transpose(1, 0, 2))
        # aux layout: [128, K] oneJ | [128, K] eqwin | [65, K] selA
        aux = np.zeros((128, 3 * K), np.float16)
        eq = labs[rot[:W]][None, :] == labs[c0:c0 + A][:, None]  # [A, W]
        k = 0
        for a in range(A):
            ga = c0 + a
            for p in range(int(s_a[ga]), int(s_a[ga] + n_a[ga])):
                j = p - (c0 - 32)
                aux[k % 128, (k // 128) * W + j] = 1.0        # oneJ
                aux[k % 128, K + (k // 128) * W:K + (k // 128) * W + W] = \
                    eq[a].astype(np.float16)                  # eqwin row
                aux[a, 2 * K + k] = 1.0                       # selA
                k += 1
        aux[A, 2 * K + k:3 * K] = 1.0   # pads select zext row 64 (+BIGW)
        maps.append({"packT": packT, "aux": aux.reshape(128, 3 * T, W)})
    return "pairs", T, maps


def run_all(labels, image_embeddings, sentence_embeddings, trace=False):
    from concourse.bass_utils import run_bass_kernel_spmd
    variant, T, maps = _prep(labels, image_embeddings, sentence_embeddings)
    nc = _get_nc(variant, T)
    res = run_bass_kernel_spmd(nc, maps, list(range(NCORES)), trace=trace)
    if variant == "pairs":
        s = c = 0.0
        for i in range(NCORES):
            x = res.results[i]["out"]
            s += float(x[:, 0:T].sum()) - float(x[:, 2 * T:3 * T].sum())
            c += float(x[:, T:2 * T].sum()) - float(x[:, 3 * T:].sum())
    else:
        parts = np.stack([res.results[i]["out"] for i in range(NCORES)])
        s = float(parts[:, 0].sum())
        c = float(parts[:, 1].sum())
    loss = np.float32(s / (c + EPS))
    return np.asarray(loss, dtype=np.float32), res


def kernel(labels, image_embeddings, sentence_embeddings):
    out, _ = run_all(labels, image_embeddings, sentence_embeddings)
    return out
